# revision 19
# baseline (speedup 1.0000x reference)
"""AttnDecoderLSTM decode step on 8 TRN2 NeuronCores, pure data-parallel.

Each core processes a 128-row batch shard (batch stays on the SBUF
partition axis end-to-end). Weights are replicated. The two large
streams (visual_context 321MB, ctx 168MB global) are read exactly once
per core using an online (no-max-subtraction) softmax: per-slice dot
products via DVE tensor_mul + ScalarE Copy(accum_out) reduce, exp on
ScalarE, and the weighted sums accumulated on TensorE as diag(e) @
slice matmuls into PSUM. Attention-score paths (which feed the alpha
softmax outputs) stay f32; the weighted sums, LSTM gates and scoring
matmuls run in bf16 (f32 PSUM accumulate) to halve TensorE time.
Dense projections use on-chip PE transposes of the activations; biases
enter as K=1 ones-row matmuls.
"""

import numpy as np

# problem dims (hardcoded per harness contract)
B, A_NUM, V_NUM, SEQ = 1024, 16, 36, 80
EMB, HID, FEAT, DOT = 256, 512, 2176, 256
P = 128
NCORES = 8
BS = B // NCORES  # 128 batch rows per core

_CACHE = {}


def _build_graph():
    from concourse import bacc, mybir
    from concourse import tile as tile_mod
    from concourse.bass import ts
    from concourse.masks import make_identity

    f32 = mybir.dt.float32
    bf16 = mybir.dt.bfloat16
    u8 = mybir.dt.uint8
    AF = mybir.ActivationFunctionType
    OP = mybir.AluOpType
    AX = mybir.AxisListType

    nc = bacc.Bacc("TRN2", target_bir_lowering=False, debug=False)

    # ---- DRAM parameters (per-core shard shapes) ----
    d_utp = nc.dram_tensor("u_t_prev", [BS, EMB], f32, kind="ExternalInput")
    d_au = nc.dram_tensor("all_u_t", [BS, A_NUM, EMB], f32, kind="ExternalInput")
    d_vc = nc.dram_tensor("visual_context", [BS, V_NUM, FEAT], f32, kind="ExternalInput")
    d_h0 = nc.dram_tensor("h_0", [BS, HID], f32, kind="ExternalInput")
    d_c0 = nc.dram_tensor("c_0", [BS, HID], f32, kind="ExternalInput")
    d_ctx = nc.dram_tensor("ctx", [BS, SEQ, HID], f32, kind="ExternalInput")
    d_mask = nc.dram_tensor("ctx_mask", [BS, SEQ], u8, kind="ExternalInput")
    d_Wvh = nc.dram_tensor("Wvh", [HID, DOT], f32, kind="ExternalInput")
    d_bvh = nc.dram_tensor("bvh", [DOT], f32, kind="ExternalInput")
    d_Wvv = nc.dram_tensor("Wvv", [FEAT, DOT], f32, kind="ExternalInput")
    d_bvv = nc.dram_tensor("bvv", [DOT], f32, kind="ExternalInput")  # softmax-invariant, unused
    d_Wih = nc.dram_tensor("W_ih", [EMB + FEAT, 4 * HID], f32, kind="ExternalInput")
    d_bih = nc.dram_tensor("b_ih", [4 * HID], f32, kind="ExternalInput")
    d_Whh = nc.dram_tensor("W_hh", [HID, 4 * HID], f32, kind="ExternalInput")
    d_bhh = nc.dram_tensor("b_hh", [4 * HID], f32, kind="ExternalInput")
    d_Win = nc.dram_tensor("W_in", [HID, HID], f32, kind="ExternalInput")
    d_Wout = nc.dram_tensor("W_out", [2 * HID, HID], f32, kind="ExternalInput")
    d_Wsh = nc.dram_tensor("Wsh", [HID, DOT], f32, kind="ExternalInput")
    d_bsh = nc.dram_tensor("bsh", [DOT], f32, kind="ExternalInput")
    d_Wsa = nc.dram_tensor("Wsa", [EMB, DOT], f32, kind="ExternalInput")
    d_bsa = nc.dram_tensor("bsa", [DOT], f32, kind="ExternalInput")
    d_Wso = nc.dram_tensor("Wso", [DOT, 1], f32, kind="ExternalInput")
    d_bso = nc.dram_tensor("bso", [1], f32, kind="ExternalInput")

    o_h1 = nc.dram_tensor("out_h1", [BS, HID], f32, kind="ExternalOutput")
    o_c1 = nc.dram_tensor("out_c1", [BS, HID], f32, kind="ExternalOutput")
    o_alpha = nc.dram_tensor("out_alpha", [BS, SEQ], f32, kind="ExternalOutput")
    o_logit = nc.dram_tensor("out_logit", [BS, A_NUM], f32, kind="ExternalOutput")
    o_alpha_v = nc.dram_tensor("out_alpha_v", [BS, V_NUM], f32, kind="ExternalOutput")

    dma = nc.sync.dma_start

    FK = FEAT // P      # 17
    HK = HID // P       # 4
    EK = EMB // P       # 2
    DK = DOT // P       # 2
    XK = (EMB + FEAT) // P  # 19
    FCH = [(c, min(512, FEAT - c)) for c in range(0, FEAT, 512)]

    with tile_mod.TileContext(nc) as tc:
        with tc.tile_pool(name="singles", bufs=1) as sg, \
             tc.tile_pool(name="psum_tp", bufs=2, space="PSUM") as pst, \
             tc.tile_pool(name="psum_mm", bufs=1, space="PSUM") as pss:

            ident = sg.tile([P, P], f32)
            make_identity(nc, ident[:])
            ones2 = sg.tile([2, P], f32)
            nc.vector.memset(ones2[:], 1.0)
            ones_row = ones2[0:1, :]

            def transp(dst_ap, src_ap):
                """dst[128,128] = src[128,128].T via PE; dst dtype sets cast."""
                pt = pst.tile([P, P], f32, tag="tpsum")
                nc.tensor.transpose(pt[:], src_ap, ident[:])
                nc.any.tensor_copy(dst_ap, pt[:])

            def transp4(dst3d, srcs):
                """Transpose up to 4 [128,128] blocks through one PSUM bank
                with a single bulk copy. dst3d[:, j, :] receives srcs[j].T."""
                for j0 in range(0, len(srcs), 4):
                    n = min(4, len(srcs) - j0)
                    pt = pst.tile([P, 4, P], f32, tag="tpsum4")
                    for j in range(n):
                        nc.tensor.transpose(pt[:, j, :], srcs[j0 + j], ident[:])
                    nc.any.tensor_copy(dst3d[:, j0:j0 + n, :], pt[:, 0:n, :])

            # ---- small inputs ----
            h0 = sg.tile([BS, HID], f32)
            dma(h0[:], d_h0[:, :])
            c0 = sg.tile([BS, HID], f32)
            dma(c0[:], d_c0[:, :])
            utp = sg.tile([BS, EMB], f32)
            dma(utp[:], d_utp[:, :])
            mask_u8 = sg.tile([BS, SEQ], u8)
            dma(mask_u8[:], d_mask[:, :])
            bvh_t = sg.tile([1, DOT], f32)
            dma(bvh_t[:], d_bvh[:].rearrange("(a n) -> a n", a=1))
            bsh_t = sg.tile([1, DOT], f32)
            dma(bsh_t[:], d_bsh[:].rearrange("(a n) -> a n", a=1))
            bsa_t = sg.tile([1, DOT], f32)
            dma(bsa_t[:], d_bsa[:].rearrange("(a n) -> a n", a=1))
            wso_t = sg.tile([1, DOT], f32)
            dma(wso_t[:], d_Wso[:, :].rearrange("n a -> a n"))
            bso_t = sg.tile([1, 1], f32)
            dma(bso_t[:], d_bso[:].rearrange("(a n) -> a n", a=1))
            bias2 = sg.tile([2, 4 * HID], f32)
            dma(bias2[0:1, :], d_bih[:].rearrange("(a n) -> a n", a=1))
            dma(bias2[1:2, :], d_bhh[:].rearrange("(a n) -> a n", a=1))

            au = sg.tile([BS, A_NUM, EMB], f32)
            dma(au[:], d_au[:, :, :])

            # maskneg[b,s] = -1e30 where masked else 0
            maskf = sg.tile([BS, SEQ], f32)
            nc.vector.tensor_copy(maskf[:], mask_u8[:])
            maskneg = sg.tile([BS, SEQ], f32)
            nc.vector.tensor_scalar_mul(maskneg[:], maskf[:], -1.0e30)

            # ---- h0T (f32 for tgt_v; bf16 copy for the W_hh term) ----
            h0T = sg.tile([P, HK, P], f32)
            transp4(h0T, [h0[:, ts(c, P)] for c in range(HK)])
            h0T_bf = sg.tile([P, HK, P], bf16)
            nc.vector.tensor_copy(h0T_bf[:], h0T[:])

            # Visual stream pools open first so the vc prefetch DMAs can
            # be issued at t=0, ahead of the setup weight traffic.
            e_v = sg.tile([BS, V_NUM], f32)
            s_v = sg.tile([BS, V_NUM], f32)
            feature = sg.tile([BS, FEAT], f32)
            VSTEP = 2
            NPRE = 3
            ctx_vis = tc.tile_pool(name="vc", bufs=3)
            pvc = ctx_vis.__enter__()
            pre_tiles = []

            # ---- proj = (h0 @ Wvh + bvh) @ Wvv.T  (f32: feeds alpha_v) ----
            proj = sg.tile([BS, FEAT], f32)
            with tc.tile_pool(name="wvvT", bufs=1) as pvT, \
                 tc.tile_pool(name="wvv_in", bufs=5) as pvi:
                # Wvv block-transposes first: their DMA+PE chain is the
                # critical path to proj, tgt_v overlaps it.
                wvvT = pvT.tile([P, DK, FK, P], f32)
                grp_tiles = {}
                for fi in range(FK):
                    wt = pvi.tile([P, DOT], f32, tag="w")
                    dma(wt[:], d_Wvv[ts(fi, P), :])
                    grp_tiles[fi] = wt
                    if fi == 3:
                        # weight DMAs for the first groups are in flight;
                        # queue the vc prefetch behind them
                        for g in range(NPRE):
                            vt = pvc.tile([BS, VSTEP, FEAT], f32, tag="vc")
                            dma(vt[:], d_vc[:, g * VSTEP:(g + 1) * VSTEP, :])
                            pre_tiles.append(vt)
                    if fi % 4 == 3 or fi == FK - 1:
                        f0 = (fi // 4) * 4
                        for dj in range(DK):
                            pt = pst.tile([P, 4, P], f32, tag="tpsum4")
                            for j in range(f0, fi + 1):
                                nc.tensor.transpose(pt[:, j - f0, :],
                                                    grp_tiles[j][:, ts(dj, P)],
                                                    ident[:])
                            nc.any.tensor_copy(wvvT[:, dj, f0:fi + 1, :],
                                               pt[:, 0:fi + 1 - f0, :])
                        grp_tiles = {}
                tgv_ps = pss.tile([BS, DOT], f32, tag="mm")
                nc.tensor.matmul(tgv_ps[:], ones_row, bvh_t[:], start=True, stop=False)
                with tc.tile_pool(name="w256", bufs=3) as w256:
                    for c in range(HK):
                        wt = w256.tile([P, DOT], f32, tag="w")
                        dma(wt[:], d_Wvh[ts(c, P), :])
                        nc.tensor.matmul(tgv_ps[:], h0T[:, c, :], wt[:],
                                         start=False, stop=(c == HK - 1))
                tgt_v = sg.tile([BS, DOT], f32)
                nc.scalar.copy(tgt_v[:], tgv_ps[:])
                tgt_vT = sg.tile([P, DK, P], f32)
                transp4(tgt_vT, [tgt_v[:, ts(c, P)] for c in range(DK)])
                with tc.tile_pool(name="psum_prj", bufs=1, space="PSUM") as psp:
                    prj_ps = psp.tile([BS, FEAT], f32, tag="prj")
                    for dj in range(DK):
                        for c0_, cw in FCH:
                            nc.tensor.matmul(
                                prj_ps[:, c0_:c0_ + cw],
                                tgt_vT[:, dj, :],
                                wvvT[:, dj].rearrange("p a b -> p (a b)")[:, c0_:c0_ + cw],
                                start=(dj == 0), stop=(dj == DK - 1))
                    nc.scalar.copy(proj[:], prj_ps[:])

            # =========================================================
            # Visual attention: one pass over visual_context. Scores in
            # f32 (DVE mul + ACT accum reduce); weighted sum in bf16 on
            # PE (diag(e_v) @ vc_v into PSUM). Casts alternate DVE/ACT.
            # =========================================================
            with tc.tile_pool(name="vcbf", bufs=2) as pvcb, \
                 tc.tile_pool(name="ttr_scr", bufs=1) as pscr, \
                 tc.tile_pool(name="diag", bufs=4) as pdg, \
                 tc.tile_pool(name="psum_acc", bufs=1, space="PSUM") as psa:
                w_ps = psa.tile([BS, FEAT], f32, tag="acc")
                scr = pscr.tile([BS, FEAT], f32, tag="scr")
                for g, v0 in enumerate(range(0, V_NUM, VSTEP)):
                    if g < NPRE:
                        vt = pre_tiles[g]
                    else:
                        vt = pvc.tile([BS, VSTEP, FEAT], f32, tag="vc")
                        dma(vt[:], d_vc[:, v0:v0 + VSTEP, :])
                    vtb = pvcb.tile([BS, VSTEP, FEAT], bf16, tag="vcb")
                    nc.scalar.copy(vtb[:], vt[:])
                    for dv in range(VSTEP):
                        v = v0 + dv
                        nc.vector.affine_mul_reduce(
                            out=scr[:], accum_out=s_v[:, v:v + 1],
                            in0=vt[:, dv, :], in1=proj[:], scale=1.0, bias=0.0)
                    nc.scalar.activation(e_v[:, v0:v0 + VSTEP],
                                         s_v[:, v0:v0 + VSTEP], AF.Exp)
                    dgs = []
                    for dv in range(VSTEP):
                        v = v0 + dv
                        dg = pdg.tile([P, P], bf16, tag="dg")
                        nc.vector.tensor_scalar_mul(dg[:], ident[:], e_v[:, v:v + 1])
                        dgs.append(dg)
                    for dv in range(VSTEP):
                        v = v0 + dv
                        for c0_, cw in FCH:
                            nc.tensor.matmul(
                                w_ps[:, c0_:c0_ + cw], dgs[dv][:],
                                vtb[:, dv, c0_:c0_ + cw],
                                start=(v == 0), stop=(v == V_NUM - 1))
                denom = sg.tile([BS, 1], f32)
                nc.vector.tensor_reduce(denom[:], e_v[:], axis=AX.X, op=OP.add)
                rden = sg.tile([BS, 1], f32)
                nc.vector.reciprocal(rden[:], denom[:])
                alpha_v = sg.tile([BS, V_NUM], f32)
                nc.vector.tensor_scalar_mul(alpha_v[:], e_v[:], rden[:])
                nc.gpsimd.dma_start(o_alpha_v[:, :], alpha_v[:])
                nc.scalar.activation(feature[:], w_ps[:], AF.Copy, scale=rden[:])
            ctx_vis.__exit__(None, None, None)

            # =========================================================
            # LSTM: gates = [utp|feature] @ W_ih + h0 @ W_hh + b (bf16)
            # =========================================================
            xT = sg.tile([P, XK, P], bf16)
            transp4(xT, [utp[:, ts(c, P)] for c in range(EK)]
                    + [feature[:, ts(c, P)] for c in range(FK)])

            ctx_wst = tc.tile_pool(name="wstage", bufs=1)
            wst = ctx_wst.__enter__()
            win_t = wst.tile([P, HK, HID], f32)
            nc.gpsimd.dma_start(win_t[:], d_Win.rearrange("(a p) n -> p a n", p=P))
            wout_t = wst.tile([P, 2 * HK, HID], f32)
            nc.gpsimd.dma_start(wout_t[:], d_Wout.rearrange("(a p) n -> p a n", p=P))
            wsh_t = wst.tile([P, HK, DOT], f32)
            nc.gpsimd.dma_start(wsh_t[:], d_Wsh.rearrange("(a p) n -> p a n", p=P))
            wsa_t = sg.tile([P, EK, DOT], f32)
            nc.gpsimd.dma_start(wsa_t[:], d_Wsa.rearrange("(a p) n -> p a n", p=P))

            GCH = [(c, 512) for c in range(0, 4 * HID, 512)]
            with tc.tile_pool(name="w2048", bufs=3) as pw, \
                 tc.tile_pool(name="w2048b", bufs=2) as pwb, \
                 tc.tile_pool(name="psum_acc2", bufs=1, space="PSUM") as psa2:
                g_ps = psa2.tile([BS, 4 * HID], f32, tag="acc")
                for c0_, cw in GCH:
                    nc.tensor.matmul(g_ps[:, c0_:c0_ + cw], ones2[:],
                                     bias2[:, c0_:c0_ + cw], start=True, stop=False)
                for k in range(XK):
                    wt = pw.tile([P, 4 * HID], f32, tag="w")
                    dma(wt[:], d_Wih[ts(k, P), :])
                    wtb = pwb.tile([P, 4 * HID], bf16, tag="wb")
                    nc.vector.tensor_copy(wtb[:], wt[:])
                    for c0_, cw in GCH:
                        nc.tensor.matmul(g_ps[:, c0_:c0_ + cw], xT[:, k, :],
                                         wtb[:, c0_:c0_ + cw], start=False, stop=False)
                for k in range(HK):
                    wt = pw.tile([P, 4 * HID], f32, tag="w")
                    dma(wt[:], d_Whh[ts(k, P), :])
                    wtb = pwb.tile([P, 4 * HID], bf16, tag="wb")
                    nc.vector.tensor_copy(wtb[:], wt[:])
                    for c0_, cw in GCH:
                        nc.tensor.matmul(g_ps[:, c0_:c0_ + cw], h0T_bf[:, k, :],
                                         wtb[:, c0_:c0_ + cw], start=False,
                                         stop=(k == HK - 1))
                sig_i = sg.tile([BS, HID], f32)
                nc.scalar.activation(sig_i[:], g_ps[:, 0:HID], AF.Tanh, scale=0.5)
                nc.vector.tensor_scalar(sig_i[:], sig_i[:], 0.5, 0.5,
                                        op0=OP.mult, op1=OP.add)
                sig_f = sg.tile([BS, HID], f32)
                nc.scalar.activation(sig_f[:], g_ps[:, HID:2 * HID], AF.Tanh, scale=0.5)
                nc.vector.tensor_scalar(sig_f[:], sig_f[:], 0.5, 0.5,
                                        op0=OP.mult, op1=OP.add)
                tanh_g = sg.tile([BS, HID], f32)
                nc.scalar.activation(tanh_g[:], g_ps[:, 2 * HID:3 * HID], AF.Tanh)
                sig_o = sg.tile([BS, HID], f32)
                nc.scalar.activation(sig_o[:], g_ps[:, 3 * HID:4 * HID], AF.Tanh, scale=0.5)
                nc.vector.tensor_scalar(sig_o[:], sig_o[:], 0.5, 0.5,
                                        op0=OP.mult, op1=OP.add)

            c1 = sg.tile([BS, HID], f32)
            nc.vector.tensor_mul(c1[:], sig_f[:], c0[:])
            ig = sg.tile([BS, HID], f32)
            nc.vector.tensor_mul(ig[:], sig_i[:], tanh_g[:])
            nc.vector.tensor_add(c1[:], c1[:], ig[:])
            nc.gpsimd.dma_start(o_c1[:, :], c1[:])
            tanh_c1 = sg.tile([BS, HID], f32)
            nc.scalar.activation(tanh_c1[:], c1[:], AF.Tanh)
            h1 = sg.tile([BS, HID], f32)
            nc.vector.tensor_mul(h1[:], sig_o[:], tanh_c1[:])
            nc.gpsimd.dma_start(o_h1[:, :], h1[:])
            h1T = sg.tile([P, HK, P], f32)
            transp4(h1T, [h1[:, ts(c, P)] for c in range(HK)])
            h1T_bf = sg.tile([P, HK, P], bf16)
            nc.vector.tensor_copy(h1T_bf[:], h1T[:])

            # ---- tgt_t = h1 @ W_in  (f32: feeds alpha) ----
            tgt_t = sg.tile([BS, HID], f32)
            tt_ps = pss.tile([BS, HID], f32, tag="mm")
            for c in range(HK):
                nc.tensor.matmul(tt_ps[:], h1T[:, c, :], win_t[:, c, :],
                                 start=(c == 0), stop=(c == HK - 1))
            nc.scalar.copy(tgt_t[:], tt_ps[:])

            # =========================================================
            # Text attention over ctx: scores f32, weighted ctx in bf16
            # on PE; mask folded into exp's bias.
            # =========================================================
            wout_bf = sg.tile([P, 2 * HK, HID], bf16)
            nc.scalar.copy(wout_bf[:], wout_t[:])
            wsh_bf = sg.tile([P, HK, DOT], bf16)
            nc.scalar.copy(wsh_bf[:], wsh_t[:])
            ctx_wst.__exit__(None, None, None)

            e_t = sg.tile([BS, SEQ], f32)
            s_t = sg.tile([BS, SEQ], f32)
            wctx = sg.tile([BS, HID], f32)
            SSTEP = 4
            with tc.tile_pool(name="ctxp", bufs=3) as pcx, \
                 tc.tile_pool(name="ctxbf", bufs=2) as pcxb, \
                 tc.tile_pool(name="ttr_scr5", bufs=1) as pscr5, \
                 tc.tile_pool(name="diag2", bufs=6) as pdg2, \
                 tc.tile_pool(name="psum_ht", bufs=1, space="PSUM") as psht, \
                 tc.tile_pool(name="psum_acc3", bufs=1, space="PSUM") as psa3:
                wc_ps = psa3.tile([BS, HID], f32, tag="acc")
                scr5 = pscr5.tile([BS, HID], f32, tag="scr")
                # h1 half of h_tilde's matmul: no text dependency, runs now
                ht_ps = psht.tile([BS, HID], f32, tag="ht")
                for c in range(HK):
                    nc.tensor.matmul(ht_ps[:], h1T_bf[:, c, :], wout_bf[:, HK + c, :],
                                     start=(c == 0), stop=False)
                for st0 in range(0, SEQ, SSTEP):
                    ct = pcx.tile([BS, SSTEP, HID], f32, tag="ctx")
                    dma(ct[:], d_ctx[:, st0:st0 + SSTEP, :])
                    ctb = pcxb.tile([BS, SSTEP, HID], bf16, tag="ctxb")
                    nc.scalar.copy(ctb[:], ct[:])
                    for dss in range(SSTEP):
                        s = st0 + dss
                        nc.vector.affine_mul_reduce(
                            out=scr5[:], accum_out=s_t[:, s:s + 1],
                            in0=ct[:, dss, :], in1=tgt_t[:], scale=1.0, bias=0.0)
                    nc.vector.tensor_add(s_t[:, st0:st0 + SSTEP],
                                         s_t[:, st0:st0 + SSTEP],
                                         maskneg[:, st0:st0 + SSTEP])
                    nc.scalar.activation(e_t[:, st0:st0 + SSTEP],
                                         s_t[:, st0:st0 + SSTEP], AF.Exp)
                    dgs = []
                    for dss in range(SSTEP):
                        s = st0 + dss
                        dg = pdg2.tile([P, P], bf16, tag="dg")
                        if dss % 2 == 0:
                            nc.vector.tensor_scalar_mul(dg[:], ident[:], e_t[:, s:s + 1])
                        else:
                            nc.scalar.mul(dg[:], ident[:], e_t[:, s:s + 1])
                        dgs.append(dg)
                    for dss in range(SSTEP):
                        s = st0 + dss
                        nc.tensor.matmul(wc_ps[:], dgs[dss][:], ctb[:, dss, :],
                                         start=(s == 0), stop=(s == SEQ - 1))
                denom_t = sg.tile([BS, 1], f32)
                nc.vector.tensor_reduce(denom_t[:], e_t[:], axis=AX.X, op=OP.add)
                rden_t = sg.tile([BS, 1], f32)
                nc.vector.reciprocal(rden_t[:], denom_t[:])
                alpha_t = sg.tile([BS, SEQ], f32)
                nc.vector.tensor_scalar_mul(alpha_t[:], e_t[:], rden_t[:])
                nc.gpsimd.dma_start(o_alpha[:, :], alpha_t[:])
                nc.scalar.activation(wctx[:], wc_ps[:], AF.Copy, scale=rden_t[:])

                # h_tilde = tanh([wctx|h1] @ W_out): wctx half joins the
                # pre-accumulated h1 half in ht_ps
                wctxT = sg.tile([P, HK, P], bf16)
                transp4(wctxT, [wctx[:, ts(c, P)] for c in range(HK)])
                h_tilde = sg.tile([BS, HID], f32)
                for c in range(HK):
                    nc.tensor.matmul(ht_ps[:], wctxT[:, c, :], wout_bf[:, c, :],
                                     start=False, stop=(c == HK - 1))
                nc.scalar.activation(h_tilde[:], ht_ps[:], AF.Tanh)

            # =========================================================
            # Action scoring (bf16 matmuls, f32 dots):
            # tw = (h_tilde @ Wsh + bsh) * wso; sw = tw @ Wsa.T
            # logit_i = au_i . sw + tw.bsa + bso
            # =========================================================
            h_tT = sg.tile([P, HK, P], bf16)
            transp4(h_tT, [h_tilde[:, ts(c, P)] for c in range(HK)])
            with tc.tile_pool(name="scr256", bufs=2) as pscr2, \
                 tc.tile_pool(name="psum_sc", bufs=3, space="PSUM") as psc:
                t_ps = psc.tile([BS, DOT], f32, tag="mm")
                nc.tensor.matmul(t_ps[:], ones_row, bsh_t[:], start=True, stop=False)
                for c in range(HK):
                    nc.tensor.matmul(t_ps[:], h_tT[:, c, :], wsh_bf[:, c, :],
                                     start=False, stop=(c == HK - 1))
                wso_ps = psc.tile([BS, DOT], f32, tag="mm")
                nc.tensor.matmul(wso_ps[:], ones_row, wso_t[:], start=True, stop=True)
                t_sb = sg.tile([BS, DOT], f32)
                nc.scalar.copy(t_sb[:], t_ps[:])
                tw = sg.tile([BS, DOT], f32)
                nc.vector.tensor_mul(tw[:], t_sb[:], wso_ps[:])
                twT = sg.tile([P, DK, P], bf16)
                transp4(twT, [tw[:, ts(c, P)] for c in range(DK)])
                wsaT = sg.tile([P, DK, EK, P], bf16)
                for dj in range(DK):
                    transp4(wsaT[:, dj], [wsa_t[:, fi, ts(dj, P)]
                                          for fi in range(EK)])
                sw_ps = psc.tile([BS, DOT], f32, tag="mm")
                for dj in range(DK):
                    nc.tensor.matmul(sw_ps[:], twT[:, dj, :],
                                     wsaT[:, dj].rearrange("p a b -> p (a b)"),
                                     start=(dj == 0), stop=(dj == DK - 1))
                sw = sg.tile([BS, DOT], f32)
                nc.scalar.copy(sw[:], sw_ps[:])
                bsa_ps = psc.tile([BS, DOT], f32, tag="mm")
                nc.tensor.matmul(bsa_ps[:], ones_row, bsa_t[:], start=True, stop=True)
                scr2 = pscr2.tile([BS, DOT], f32, tag="scr")
                bsa_sb = sg.tile([BS, DOT], f32)
                nc.scalar.copy(bsa_sb[:], bsa_ps[:])
                tb = sg.tile([BS, 1], f32)
                nc.vector.affine_mul_reduce(
                    out=scr2[:], accum_out=tb[:], in0=tw[:], in1=bsa_sb[:],
                    scale=1.0, bias=0.0)
                bso_ps = psc.tile([BS, 1], f32, tag="mm")
                nc.tensor.matmul(bso_ps[:], ones_row, bso_t[:], start=True, stop=True)
                nc.vector.tensor_add(tb[:], tb[:], bso_ps[:])
                lg = sg.tile([BS, A_NUM], f32)
                for i in range(A_NUM):
                    nc.vector.affine_mul_reduce(
                        out=scr2[:], accum_out=lg[:, i:i + 1],
                        in0=au[:, i, :], in1=sw[:], scale=1.0, bias=0.0)
                logit = sg.tile([BS, A_NUM], f32)
                nc.vector.tensor_scalar_add(logit[:], lg[:], tb[:])
                nc.gpsimd.dma_start(o_logit[:, :], logit[:])

    nc.compile()
    return nc


def _get_graph():
    if "nc" not in _CACHE:
        _CACHE["nc"] = _build_graph()
    return _CACHE["nc"]


def _run(inputs, trace=False):
    from concourse.bass_utils import run_bass_kernel_spmd

    nc = _get_graph()
    arr = {k: np.asarray(v) for k, v in inputs.items()}
    sharded = {"u_t_prev", "all_u_t", "visual_context", "h_0", "c_0", "ctx", "ctx_mask"}
    in_maps = []
    for i in range(NCORES):
        m = {}
        for k, v in arr.items():
            if k == "ctx_mask":
                v = v.astype(np.uint8)
            else:
                v = v.astype(np.float32, copy=False)
            m[k] = v[i * BS:(i + 1) * BS] if k in sharded else v
        in_maps.append(m)
    res = run_bass_kernel_spmd(nc, in_maps, core_ids=list(range(NCORES)), trace=trace)
    outs = res.results
    h1 = np.concatenate([outs[i]["out_h1"] for i in range(NCORES)], axis=0)
    c1 = np.concatenate([outs[i]["out_c1"] for i in range(NCORES)], axis=0)
    alpha = np.concatenate([outs[i]["out_alpha"] for i in range(NCORES)], axis=0)
    logit = np.concatenate([outs[i]["out_logit"] for i in range(NCORES)], axis=0)
    alpha_v = np.concatenate([outs[i]["out_alpha_v"] for i in range(NCORES)], axis=0)
    return (h1, c1, alpha, logit, alpha_v), res


def kernel(**inputs):
    out, _ = _run(inputs, trace=False)
    return out


# revision 20
# speedup vs baseline: 1.1003x; 1.1003x over previous
"""AttnDecoderLSTM decode step on 8 TRN2 NeuronCores, pure data-parallel.

Each core processes a 128-row batch shard (batch stays on the SBUF
partition axis end-to-end). Weights are replicated. The two large
streams (visual_context 321MB, ctx 168MB global) are read exactly once
per core using an online (no-max-subtraction) softmax: per-slice dot
products via DVE tensor_mul + ScalarE Copy(accum_out) reduce, exp on
ScalarE, and the weighted sums accumulated on TensorE as diag(e) @
slice matmuls into PSUM. Attention-score paths (which feed the alpha
softmax outputs) stay f32; the weighted sums, LSTM gates and scoring
matmuls run in bf16 (f32 PSUM accumulate) to halve TensorE time.
Dense projections use on-chip PE transposes of the activations; biases
enter as K=1 ones-row matmuls.
"""

import numpy as np

# problem dims (hardcoded per harness contract)
B, A_NUM, V_NUM, SEQ = 1024, 16, 36, 80
EMB, HID, FEAT, DOT = 256, 512, 2176, 256
P = 128
NCORES = 8
BS = B // NCORES  # 128 batch rows per core

_CACHE = {}


def _build_graph():
    from concourse import bacc, mybir
    from concourse import tile as tile_mod
    from concourse.bass import ts
    from concourse.masks import make_identity

    f32 = mybir.dt.float32
    bf16 = mybir.dt.bfloat16
    u8 = mybir.dt.uint8
    AF = mybir.ActivationFunctionType
    OP = mybir.AluOpType
    AX = mybir.AxisListType

    nc = bacc.Bacc("TRN2", target_bir_lowering=False, debug=False)

    # ---- DRAM parameters (per-core shard shapes) ----
    d_utp = nc.dram_tensor("u_t_prev", [BS, EMB], f32, kind="ExternalInput")
    d_au = nc.dram_tensor("all_u_t", [BS, A_NUM, EMB], f32, kind="ExternalInput")
    d_vc = nc.dram_tensor("visual_context", [BS, V_NUM, FEAT], f32, kind="ExternalInput")
    d_h0 = nc.dram_tensor("h_0", [BS, HID], f32, kind="ExternalInput")
    d_c0 = nc.dram_tensor("c_0", [BS, HID], f32, kind="ExternalInput")
    d_ctx = nc.dram_tensor("ctx", [BS, SEQ, HID], f32, kind="ExternalInput")
    d_mask = nc.dram_tensor("ctx_mask", [BS, SEQ], u8, kind="ExternalInput")
    d_Wvh = nc.dram_tensor("Wvh", [HID, DOT], f32, kind="ExternalInput")
    d_bvh = nc.dram_tensor("bvh", [DOT], f32, kind="ExternalInput")
    d_Wvv = nc.dram_tensor("Wvv", [FEAT, DOT], f32, kind="ExternalInput")
    d_bvv = nc.dram_tensor("bvv", [DOT], f32, kind="ExternalInput")  # softmax-invariant, unused
    d_Wih = nc.dram_tensor("W_ih", [EMB + FEAT, 4 * HID], f32, kind="ExternalInput")
    d_bih = nc.dram_tensor("b_ih", [4 * HID], f32, kind="ExternalInput")
    d_Whh = nc.dram_tensor("W_hh", [HID, 4 * HID], f32, kind="ExternalInput")
    d_bhh = nc.dram_tensor("b_hh", [4 * HID], f32, kind="ExternalInput")
    d_Win = nc.dram_tensor("W_in", [HID, HID], f32, kind="ExternalInput")
    d_Wout = nc.dram_tensor("W_out", [2 * HID, HID], f32, kind="ExternalInput")
    d_Wsh = nc.dram_tensor("Wsh", [HID, DOT], f32, kind="ExternalInput")
    d_bsh = nc.dram_tensor("bsh", [DOT], f32, kind="ExternalInput")
    d_Wsa = nc.dram_tensor("Wsa", [EMB, DOT], f32, kind="ExternalInput")
    d_bsa = nc.dram_tensor("bsa", [DOT], f32, kind="ExternalInput")
    d_Wso = nc.dram_tensor("Wso", [DOT, 1], f32, kind="ExternalInput")
    d_bso = nc.dram_tensor("bso", [1], f32, kind="ExternalInput")

    o_h1 = nc.dram_tensor("out_h1", [BS, HID], f32, kind="ExternalOutput")
    o_c1 = nc.dram_tensor("out_c1", [BS, HID], f32, kind="ExternalOutput")
    o_alpha = nc.dram_tensor("out_alpha", [BS, SEQ], f32, kind="ExternalOutput")
    o_logit = nc.dram_tensor("out_logit", [BS, A_NUM], f32, kind="ExternalOutput")
    o_alpha_v = nc.dram_tensor("out_alpha_v", [BS, V_NUM], f32, kind="ExternalOutput")

    dma = nc.sync.dma_start

    FK = FEAT // P      # 17
    HK = HID // P       # 4
    EK = EMB // P       # 2
    DK = DOT // P       # 2
    XK = (EMB + FEAT) // P  # 19
    FCH = [(c, min(512, FEAT - c)) for c in range(0, FEAT, 512)]

    with tile_mod.TileContext(nc) as tc:
        with tc.tile_pool(name="singles", bufs=1) as sg, \
             tc.tile_pool(name="psum_tp", bufs=2, space="PSUM") as pst, \
             tc.tile_pool(name="psum_mm", bufs=1, space="PSUM") as pss:

            ident = sg.tile([P, P], f32)
            make_identity(nc, ident[:])
            ones2 = sg.tile([2, P], f32)
            nc.vector.memset(ones2[:], 1.0)
            ones_row = ones2[0:1, :]

            def transp(dst_ap, src_ap):
                """dst[128,128] = src[128,128].T via PE; dst dtype sets cast."""
                pt = pst.tile([P, P], f32, tag="tpsum")
                nc.tensor.transpose(pt[:], src_ap, ident[:])
                nc.any.tensor_copy(dst_ap, pt[:])

            def transp4(dst3d, srcs):
                """Transpose up to 4 [128,128] blocks through one PSUM bank
                with a single bulk copy. dst3d[:, j, :] receives srcs[j].T."""
                for j0 in range(0, len(srcs), 4):
                    n = min(4, len(srcs) - j0)
                    pt = pst.tile([P, 4, P], f32, tag="tpsum4")
                    for j in range(n):
                        nc.tensor.transpose(pt[:, j, :], srcs[j0 + j], ident[:])
                    nc.any.tensor_copy(dst3d[:, j0:j0 + n, :], pt[:, 0:n, :])

            # ---- small inputs ----
            h0 = sg.tile([BS, HID], f32)
            dma(h0[:], d_h0[:, :])
            c0 = sg.tile([BS, HID], f32)
            dma(c0[:], d_c0[:, :])
            utp = sg.tile([BS, EMB], f32)
            dma(utp[:], d_utp[:, :])
            mask_u8 = sg.tile([BS, SEQ], u8)
            dma(mask_u8[:], d_mask[:, :])
            bvh_t = sg.tile([1, DOT], f32)
            dma(bvh_t[:], d_bvh[:].rearrange("(a n) -> a n", a=1))
            bsh_t = sg.tile([1, DOT], f32)
            dma(bsh_t[:], d_bsh[:].rearrange("(a n) -> a n", a=1))
            bsa_t = sg.tile([1, DOT], f32)
            dma(bsa_t[:], d_bsa[:].rearrange("(a n) -> a n", a=1))
            wso_t = sg.tile([1, DOT], f32)
            dma(wso_t[:], d_Wso[:, :].rearrange("n a -> a n"))
            bso_t = sg.tile([1, 1], f32)
            dma(bso_t[:], d_bso[:].rearrange("(a n) -> a n", a=1))
            bias2 = sg.tile([2, 4 * HID], f32)
            dma(bias2[0:1, :], d_bih[:].rearrange("(a n) -> a n", a=1))
            dma(bias2[1:2, :], d_bhh[:].rearrange("(a n) -> a n", a=1))

            au = sg.tile([BS, A_NUM, EMB], f32)
            dma(au[:], d_au[:, :, :])

            # maskneg[b,s] = -1e30 where masked else 0
            maskf = sg.tile([BS, SEQ], f32)
            nc.vector.tensor_copy(maskf[:], mask_u8[:])
            maskneg = sg.tile([BS, SEQ], f32)
            nc.vector.tensor_scalar_mul(maskneg[:], maskf[:], -1.0e30)

            # ---- h0T (f32 for tgt_v; bf16 copy for the W_hh term) ----
            h0T = sg.tile([P, HK, P], f32)
            transp4(h0T, [h0[:, ts(c, P)] for c in range(HK)])
            h0T_bf = sg.tile([P, HK, P], bf16)
            nc.vector.tensor_copy(h0T_bf[:], h0T[:])

            # Visual stream pools open first so the vc prefetch DMAs can
            # be issued at t=0, ahead of the setup weight traffic.
            e_v = sg.tile([BS, V_NUM], f32)
            s_v = sg.tile([BS, V_NUM], f32)
            feature = sg.tile([BS, FEAT], f32)
            VSTEP = 2
            NPRE = 3
            ctx_vis = tc.tile_pool(name="vc", bufs=3)
            pvc = ctx_vis.__enter__()
            pre_tiles = []

            # ---- proj = (h0 @ Wvh + bvh) @ Wvv.T  (f32: feeds alpha_v) ----
            proj = sg.tile([BS, FEAT], f32)
            with tc.tile_pool(name="wvvT", bufs=1) as pvT, \
                 tc.tile_pool(name="wvv_in", bufs=5) as pvi:
                # Wvv block-transposes first: their DMA+PE chain is the
                # critical path to proj, tgt_v overlaps it.
                wvvT = pvT.tile([P, DK, FK, P], f32)
                grp_tiles = {}
                for fi in range(FK):
                    wt = pvi.tile([P, DOT], f32, tag="w")
                    dma(wt[:], d_Wvv[ts(fi, P), :])
                    grp_tiles[fi] = wt
                    if fi == 3:
                        # weight DMAs for the first groups are in flight;
                        # queue the vc prefetch behind them
                        for g in range(NPRE):
                            vt = pvc.tile([BS, VSTEP, FEAT], f32, tag="vc")
                            dma(vt[:], d_vc[:, g * VSTEP:(g + 1) * VSTEP, :])
                            pre_tiles.append(vt)
                    if fi % 4 == 3 or fi == FK - 1:
                        f0 = (fi // 4) * 4
                        for dj in range(DK):
                            pt = pst.tile([P, 4, P], f32, tag="tpsum4")
                            for j in range(f0, fi + 1):
                                nc.tensor.transpose(pt[:, j - f0, :],
                                                    grp_tiles[j][:, ts(dj, P)],
                                                    ident[:])
                            nc.any.tensor_copy(wvvT[:, dj, f0:fi + 1, :],
                                               pt[:, 0:fi + 1 - f0, :])
                        grp_tiles = {}
                tgv_ps = pss.tile([BS, DOT], f32, tag="mm")
                nc.tensor.matmul(tgv_ps[:], ones_row, bvh_t[:], start=True, stop=False)
                with tc.tile_pool(name="w256", bufs=3) as w256:
                    for c in range(HK):
                        wt = w256.tile([P, DOT], f32, tag="w")
                        dma(wt[:], d_Wvh[ts(c, P), :])
                        nc.tensor.matmul(tgv_ps[:], h0T[:, c, :], wt[:],
                                         start=False, stop=(c == HK - 1))
                tgt_v = sg.tile([BS, DOT], f32)
                nc.scalar.copy(tgt_v[:], tgv_ps[:])
                tgt_vT = sg.tile([P, DK, P], f32)
                transp4(tgt_vT, [tgt_v[:, ts(c, P)] for c in range(DK)])
                with tc.tile_pool(name="psum_prj", bufs=1, space="PSUM") as psp:
                    prj_ps = psp.tile([BS, FEAT], f32, tag="prj")
                    for dj in range(DK):
                        for c0_, cw in FCH:
                            nc.tensor.matmul(
                                prj_ps[:, c0_:c0_ + cw],
                                tgt_vT[:, dj, :],
                                wvvT[:, dj].rearrange("p a b -> p (a b)")[:, c0_:c0_ + cw],
                                start=(dj == 0), stop=(dj == DK - 1))
                    nc.scalar.copy(proj[:], prj_ps[:])

            # =========================================================
            # Visual attention: one pass over visual_context. Scores in
            # f32 (DVE mul + ACT accum reduce); weighted sum in bf16 on
            # PE (diag(e_v) @ vc_v into PSUM). Casts alternate DVE/ACT.
            # =========================================================
            with tc.tile_pool(name="vcbf", bufs=2) as pvcb, \
                 tc.tile_pool(name="ttr_scr", bufs=1) as pscr, \
                 tc.tile_pool(name="diag", bufs=4) as pdg, \
                 tc.tile_pool(name="psum_acc", bufs=1, space="PSUM") as psa:
                w_ps = psa.tile([BS, FEAT], f32, tag="acc")
                scr = pscr.tile([BS, FEAT], f32, tag="scr")
                for g, v0 in enumerate(range(0, V_NUM, VSTEP)):
                    if g < NPRE:
                        vt = pre_tiles[g]
                    else:
                        vt = pvc.tile([BS, VSTEP, FEAT], f32, tag="vc")
                        dma(vt[:], d_vc[:, v0:v0 + VSTEP, :])
                    vtb = pvcb.tile([BS, VSTEP, FEAT], bf16, tag="vcb")
                    nc.scalar.copy(vtb[:], vt[:])
                    for dv in range(VSTEP):
                        v = v0 + dv
                        nc.vector.affine_mul_reduce(
                            out=scr[:], accum_out=s_v[:, v:v + 1],
                            in0=vt[:, dv, :], in1=proj[:], scale=1.0, bias=0.0)
                    nc.scalar.activation(e_v[:, v0:v0 + VSTEP],
                                         s_v[:, v0:v0 + VSTEP], AF.Exp)
                    dgs = []
                    for dv in range(VSTEP):
                        v = v0 + dv
                        dg = pdg.tile([P, P], bf16, tag="dg")
                        nc.vector.tensor_scalar_mul(dg[:], ident[:], e_v[:, v:v + 1])
                        dgs.append(dg)
                    for dv in range(VSTEP):
                        v = v0 + dv
                        for c0_, cw in FCH:
                            nc.tensor.matmul(
                                w_ps[:, c0_:c0_ + cw], dgs[dv][:],
                                vtb[:, dv, c0_:c0_ + cw],
                                start=(v == 0), stop=(v == V_NUM - 1))
                denom = sg.tile([BS, 1], f32)
                nc.vector.tensor_reduce(denom[:], e_v[:], axis=AX.X, op=OP.add)
                rden = sg.tile([BS, 1], f32)
                nc.vector.reciprocal(rden[:], denom[:])
                alpha_v = sg.tile([BS, V_NUM], f32)
                nc.vector.tensor_scalar_mul(alpha_v[:], e_v[:], rden[:])
                nc.gpsimd.dma_start(o_alpha_v[:, :], alpha_v[:])
                nc.scalar.activation(feature[:], w_ps[:], AF.Copy, scale=rden[:])
            ctx_vis.__exit__(None, None, None)

            # =========================================================
            # LSTM: gates = [utp|feature] @ W_ih + h0 @ W_hh + b (bf16)
            # =========================================================
            xT = sg.tile([P, XK, P], bf16)
            transp4(xT, [utp[:, ts(c, P)] for c in range(EK)]
                    + [feature[:, ts(c, P)] for c in range(FK)])

            ctx_wst = tc.tile_pool(name="wstage", bufs=1)
            wst = ctx_wst.__enter__()
            win_t = wst.tile([P, HK, HID], f32)
            wout_t = wst.tile([P, 2 * HK, HID], f32)
            wsh_t = wst.tile([P, HK, DOT], f32)
            wsa_t = wst.tile([P, EK, DOT], f32)

            GCH = [(c, 512) for c in range(0, 4 * HID, 512)]
            with tc.tile_pool(name="w2048", bufs=3) as pw, \
                 tc.tile_pool(name="w2048b", bufs=2) as pwb, \
                 tc.tile_pool(name="psum_acc2", bufs=1, space="PSUM") as psa2:
                g_ps = psa2.tile([BS, 4 * HID], f32, tag="acc")
                for c0_, cw in GCH:
                    nc.tensor.matmul(g_ps[:, c0_:c0_ + cw], ones2[:],
                                     bias2[:, c0_:c0_ + cw], start=True, stop=False)
                for k in range(XK):
                    if k == 10:
                        nc.gpsimd.dma_start(
                            win_t[:], d_Win.rearrange("(a p) n -> p a n", p=P))
                    wt = pw.tile([P, 4 * HID], f32, tag="w")
                    dma(wt[:], d_Wih[ts(k, P), :])
                    wtb = pwb.tile([P, 4 * HID], bf16, tag="wb")
                    nc.vector.tensor_copy(wtb[:], wt[:])
                    for c0_, cw in GCH:
                        nc.tensor.matmul(g_ps[:, c0_:c0_ + cw], xT[:, k, :],
                                         wtb[:, c0_:c0_ + cw], start=False, stop=False)
                for k in range(HK):
                    wt = pw.tile([P, 4 * HID], f32, tag="w")
                    dma(wt[:], d_Whh[ts(k, P), :])
                    wtb = pwb.tile([P, 4 * HID], bf16, tag="wb")
                    nc.vector.tensor_copy(wtb[:], wt[:])
                    for c0_, cw in GCH:
                        nc.tensor.matmul(g_ps[:, c0_:c0_ + cw], h0T_bf[:, k, :],
                                         wtb[:, c0_:c0_ + cw], start=False,
                                         stop=(k == HK - 1))
                sig_i = sg.tile([BS, HID], f32)
                nc.scalar.activation(sig_i[:], g_ps[:, 0:HID], AF.Tanh, scale=0.5)
                nc.vector.tensor_scalar(sig_i[:], sig_i[:], 0.5, 0.5,
                                        op0=OP.mult, op1=OP.add)
                sig_f = sg.tile([BS, HID], f32)
                nc.scalar.activation(sig_f[:], g_ps[:, HID:2 * HID], AF.Tanh, scale=0.5)
                nc.vector.tensor_scalar(sig_f[:], sig_f[:], 0.5, 0.5,
                                        op0=OP.mult, op1=OP.add)
                tanh_g = sg.tile([BS, HID], f32)
                nc.scalar.activation(tanh_g[:], g_ps[:, 2 * HID:3 * HID], AF.Tanh)
                sig_o = sg.tile([BS, HID], f32)
                nc.scalar.activation(sig_o[:], g_ps[:, 3 * HID:4 * HID], AF.Tanh, scale=0.5)
                nc.vector.tensor_scalar(sig_o[:], sig_o[:], 0.5, 0.5,
                                        op0=OP.mult, op1=OP.add)

            c1 = sg.tile([BS, HID], f32)
            nc.vector.tensor_mul(c1[:], sig_f[:], c0[:])
            ig = sg.tile([BS, HID], f32)
            nc.vector.tensor_mul(ig[:], sig_i[:], tanh_g[:])
            nc.vector.tensor_add(c1[:], c1[:], ig[:])
            nc.gpsimd.dma_start(o_c1[:, :], c1[:])
            tanh_c1 = sg.tile([BS, HID], f32)
            nc.scalar.activation(tanh_c1[:], c1[:], AF.Tanh)
            h1 = sg.tile([BS, HID], f32)
            nc.vector.tensor_mul(h1[:], sig_o[:], tanh_c1[:])
            nc.gpsimd.dma_start(o_h1[:, :], h1[:])
            h1T = sg.tile([P, HK, P], f32)
            transp4(h1T, [h1[:, ts(c, P)] for c in range(HK)])
            h1T_bf = sg.tile([P, HK, P], bf16)
            nc.vector.tensor_copy(h1T_bf[:], h1T[:])

            # ---- tgt_t = h1 @ W_in  (f32: feeds alpha) ----
            tgt_t = sg.tile([BS, HID], f32)
            tt_ps = pss.tile([BS, HID], f32, tag="mm")
            for c in range(HK):
                nc.tensor.matmul(tt_ps[:], h1T[:, c, :], win_t[:, c, :],
                                 start=(c == 0), stop=(c == HK - 1))
            nc.scalar.copy(tgt_t[:], tt_ps[:])

            # =========================================================
            # Text attention over ctx: scores f32, weighted ctx in bf16
            # on PE; mask folded into exp's bias.
            # =========================================================
            nc.gpsimd.dma_start(wout_t[:], d_Wout.rearrange("(a p) n -> p a n", p=P))
            nc.gpsimd.dma_start(wsh_t[:], d_Wsh.rearrange("(a p) n -> p a n", p=P))
            nc.gpsimd.dma_start(wsa_t[:], d_Wsa.rearrange("(a p) n -> p a n", p=P))

            e_t = sg.tile([BS, SEQ], f32)
            s_t = sg.tile([BS, SEQ], f32)
            wctx = sg.tile([BS, HID], f32)
            SSTEP = 4
            with tc.tile_pool(name="ctxp", bufs=3) as pcx, \
                 tc.tile_pool(name="ctxbf", bufs=2) as pcxb, \
                 tc.tile_pool(name="ttr_scr5", bufs=1) as pscr5, \
                 tc.tile_pool(name="diag2", bufs=6) as pdg2, \
                 tc.tile_pool(name="psum_ht", bufs=1, space="PSUM") as psht, \
                 tc.tile_pool(name="psum_acc3", bufs=1, space="PSUM") as psa3:
                wc_ps = psa3.tile([BS, HID], f32, tag="acc")
                scr5 = pscr5.tile([BS, HID], f32, tag="scr")
                # h1 half of h_tilde's matmul: no text dependency, runs now
                ht_ps = psht.tile([BS, HID], f32, tag="ht")
                for c in range(HK):
                    nc.tensor.matmul(ht_ps[:], h1T[:, c, :], wout_t[:, HK + c, :],
                                     start=(c == 0), stop=False)
                for st0 in range(0, SEQ, SSTEP):
                    ct = pcx.tile([BS, SSTEP, HID], f32, tag="ctx")
                    dma(ct[:], d_ctx[:, st0:st0 + SSTEP, :])
                    ctb = pcxb.tile([BS, SSTEP, HID], bf16, tag="ctxb")
                    nc.scalar.copy(ctb[:], ct[:])
                    for dss in range(SSTEP):
                        s = st0 + dss
                        nc.vector.affine_mul_reduce(
                            out=scr5[:], accum_out=s_t[:, s:s + 1],
                            in0=ct[:, dss, :], in1=tgt_t[:], scale=1.0, bias=0.0)
                    nc.vector.tensor_add(s_t[:, st0:st0 + SSTEP],
                                         s_t[:, st0:st0 + SSTEP],
                                         maskneg[:, st0:st0 + SSTEP])
                    nc.scalar.activation(e_t[:, st0:st0 + SSTEP],
                                         s_t[:, st0:st0 + SSTEP], AF.Exp)
                    dgs = []
                    for dss in range(SSTEP):
                        s = st0 + dss
                        dg = pdg2.tile([P, P], bf16, tag="dg")
                        if dss % 2 == 0:
                            nc.vector.tensor_scalar_mul(dg[:], ident[:], e_t[:, s:s + 1])
                        else:
                            nc.scalar.mul(dg[:], ident[:], e_t[:, s:s + 1])
                        dgs.append(dg)
                    for dss in range(SSTEP):
                        s = st0 + dss
                        nc.tensor.matmul(wc_ps[:], dgs[dss][:], ctb[:, dss, :],
                                         start=(s == 0), stop=(s == SEQ - 1))
                denom_t = sg.tile([BS, 1], f32)
                nc.vector.tensor_reduce(denom_t[:], e_t[:], axis=AX.X, op=OP.add)
                rden_t = sg.tile([BS, 1], f32)
                nc.vector.reciprocal(rden_t[:], denom_t[:])
                alpha_t = sg.tile([BS, SEQ], f32)
                nc.vector.tensor_scalar_mul(alpha_t[:], e_t[:], rden_t[:])
                nc.gpsimd.dma_start(o_alpha[:, :], alpha_t[:])
                nc.scalar.activation(wctx[:], wc_ps[:], AF.Copy, scale=rden_t[:])

                # h_tilde = tanh([wctx|h1] @ W_out): wctx half joins the
                # pre-accumulated h1 half in ht_ps
                wctxT = sg.tile([P, HK, P], f32)
                transp4(wctxT, [wctx[:, ts(c, P)] for c in range(HK)])
                h_tilde = sg.tile([BS, HID], f32)
                for c in range(HK):
                    nc.tensor.matmul(ht_ps[:], wctxT[:, c, :], wout_t[:, c, :],
                                     start=False, stop=(c == HK - 1))
                nc.scalar.activation(h_tilde[:], ht_ps[:], AF.Tanh)

            # =========================================================
            # Action scoring (bf16 matmuls, f32 dots):
            # tw = (h_tilde @ Wsh + bsh) * wso; sw = tw @ Wsa.T
            # logit_i = au_i . sw + tw.bsa + bso
            # =========================================================
            h_tT = sg.tile([P, HK, P], f32)
            transp4(h_tT, [h_tilde[:, ts(c, P)] for c in range(HK)])
            with tc.tile_pool(name="scr256", bufs=2) as pscr2, \
                 tc.tile_pool(name="psum_sc", bufs=3, space="PSUM") as psc:
                t_ps = psc.tile([BS, DOT], f32, tag="mm")
                nc.tensor.matmul(t_ps[:], ones_row, bsh_t[:], start=True, stop=False)
                for c in range(HK):
                    nc.tensor.matmul(t_ps[:], h_tT[:, c, :], wsh_t[:, c, :],
                                     start=False, stop=(c == HK - 1))
                wso_ps = psc.tile([BS, DOT], f32, tag="mm")
                nc.tensor.matmul(wso_ps[:], ones_row, wso_t[:], start=True, stop=True)
                t_sb = sg.tile([BS, DOT], f32)
                nc.scalar.copy(t_sb[:], t_ps[:])
                tw = sg.tile([BS, DOT], f32)
                nc.vector.tensor_mul(tw[:], t_sb[:], wso_ps[:])
                twT = sg.tile([P, DK, P], f32)
                transp4(twT, [tw[:, ts(c, P)] for c in range(DK)])
                wsaT = sg.tile([P, DK, EK, P], f32)
                for dj in range(DK):
                    transp4(wsaT[:, dj], [wsa_t[:, fi, ts(dj, P)]
                                          for fi in range(EK)])
                sw_ps = psc.tile([BS, DOT], f32, tag="mm")
                for dj in range(DK):
                    nc.tensor.matmul(sw_ps[:], twT[:, dj, :],
                                     wsaT[:, dj].rearrange("p a b -> p (a b)"),
                                     start=(dj == 0), stop=(dj == DK - 1))
                sw = sg.tile([BS, DOT], f32)
                nc.scalar.copy(sw[:], sw_ps[:])
                bsa_ps = psc.tile([BS, DOT], f32, tag="mm")
                nc.tensor.matmul(bsa_ps[:], ones_row, bsa_t[:], start=True, stop=True)
                scr2 = pscr2.tile([BS, DOT], f32, tag="scr")
                bsa_sb = sg.tile([BS, DOT], f32)
                nc.scalar.copy(bsa_sb[:], bsa_ps[:])
                tb = sg.tile([BS, 1], f32)
                nc.vector.affine_mul_reduce(
                    out=scr2[:], accum_out=tb[:], in0=tw[:], in1=bsa_sb[:],
                    scale=1.0, bias=0.0)
                bso_ps = psc.tile([BS, 1], f32, tag="mm")
                nc.tensor.matmul(bso_ps[:], ones_row, bso_t[:], start=True, stop=True)
                nc.vector.tensor_add(tb[:], tb[:], bso_ps[:])
                lg = sg.tile([BS, A_NUM], f32)
                for i in range(A_NUM):
                    nc.vector.affine_mul_reduce(
                        out=scr2[:], accum_out=lg[:, i:i + 1],
                        in0=au[:, i, :], in1=sw[:], scale=1.0, bias=0.0)
                logit = sg.tile([BS, A_NUM], f32)
                nc.vector.tensor_scalar_add(logit[:], lg[:], tb[:])
                nc.gpsimd.dma_start(o_logit[:, :], logit[:])
            ctx_wst.__exit__(None, None, None)

    nc.compile()
    return nc


def _get_graph():
    if "nc" not in _CACHE:
        _CACHE["nc"] = _build_graph()
    return _CACHE["nc"]


def _run(inputs, trace=False):
    from concourse.bass_utils import run_bass_kernel_spmd

    nc = _get_graph()
    arr = {k: np.asarray(v) for k, v in inputs.items()}
    sharded = {"u_t_prev", "all_u_t", "visual_context", "h_0", "c_0", "ctx", "ctx_mask"}
    in_maps = []
    for i in range(NCORES):
        m = {}
        for k, v in arr.items():
            if k == "ctx_mask":
                v = v.astype(np.uint8)
            else:
                v = v.astype(np.float32, copy=False)
            m[k] = v[i * BS:(i + 1) * BS] if k in sharded else v
        in_maps.append(m)
    res = run_bass_kernel_spmd(nc, in_maps, core_ids=list(range(NCORES)), trace=trace)
    outs = res.results
    h1 = np.concatenate([outs[i]["out_h1"] for i in range(NCORES)], axis=0)
    c1 = np.concatenate([outs[i]["out_c1"] for i in range(NCORES)], axis=0)
    alpha = np.concatenate([outs[i]["out_alpha"] for i in range(NCORES)], axis=0)
    logit = np.concatenate([outs[i]["out_logit"] for i in range(NCORES)], axis=0)
    alpha_v = np.concatenate([outs[i]["out_alpha_v"] for i in range(NCORES)], axis=0)
    return (h1, c1, alpha, logit, alpha_v), res


def kernel(**inputs):
    out, _ = _run(inputs, trace=False)
    return out
